# revision 2
# baseline (speedup 1.0000x reference)
"""BitNet MLP (ternary weights + int8 per-token activations) on 8 TRN2 NeuronCores.

Strategy: data-parallel over tokens (2048 tokens/core). Each core:
  P0: partial sum(|w|) over its 1/8 weight shard -> tiny AllReduce -> per-tensor
      weight scales (matches jnp.mean(|w|) + EPS).
  A:  per-token absmax of x (transposed layout, partition_all_reduce over H) ->
      xqT = round(xT / s_x) as bf16, resident in SBUF.
  B:  stream gate/up weights fp32 (host-packed [K,M] tiles), quantize to ternary
      bf16 on the fly, matmul (K=H on partitions) -> psum [i, tok]; dequant,
      silu(gate)*up -> h fp32 -> DRAM scratch; track per-token absmax of h.
  C0: stream w_down^T fp32, quantize -> wdq bf16 -> DRAM scratch.
  C:  per 512-token group: re-read h, quantize -> hq bf16 (stationary), stream
      wdq tiles (moving), matmul -> psum [tok, h]; dequant -> y.

All quantized matmuls are exact: bf16 holds ints <= 256 exactly, fp32 PSUM
accumulates integer partial sums < 2^24 exactly. round() uses the magic-number
trick (x + 1.5*2^23 - 1.5*2^23) = round-half-to-even, matching jnp.round.
"""

from contextlib import ExitStack

import numpy as np

import concourse.bass as bass
import concourse.bass_isa as bass_isa
import concourse.mybir as mybir
import concourse.tile as tile
from concourse import bacc
from concourse.bass import ts as bts
from concourse.bass_utils import run_bass_kernel_spmd

NCORES = 8
EPS = 1e-5
MAGIC = 12582912.0  # 1.5 * 2^23; x+MAGIC-MAGIC == round-half-even(x) for |x|<2^22

F32 = mybir.dt.float32
BF16 = mybir.dt.bfloat16
ALU = mybir.AluOpType
AX = mybir.AxisListType


def build_nc(TPC, H, I, single=False, phases="ABC", reps=1, act_round=False, p0_in_body=False, p0_bufs=4):
    """Build the per-core Bass program. TPC tokens/core, hidden H, intermediate I."""
    P = 128
    HT = H // P          # h tiles (contraction tiles for gate/up)
    IT = I // P          # i tiles
    NB = TPC // 512      # 512-token blocks (B) and groups (C)
    SH_ELEMS = (I * H) // NCORES          # weight-shard elements per core (per tensor)
    SH_F = SH_ELEMS // P                  # free size of [128, SH_F] shard view
    NCH = (SH_F + 2047) // 2048           # 2048-wide chunks
    assert SH_F % 128 == 0 and SH_F % NCH == 0
    CHF = SH_F // NCH                     # chunk free size
    INV_CNT = 1.0 / float(I * H)          # exact power of two for our sizes

    nc = bacc.Bacc("TRN2", target_bir_lowering=False, debug=False,
               num_devices=(1 if single else NCORES))

    xT = nc.dram_tensor("xT", [H, TPC], F32, kind="ExternalInput")
    wg = nc.dram_tensor("wg", [IT, P, HT, P], F32, kind="ExternalInput")
    wu = nc.dram_tensor("wu", [IT, P, HT, P], F32, kind="ExternalInput")
    wdT = nc.dram_tensor("wdT", [I, H], F32, kind="ExternalInput")
    wsh = nc.dram_tensor("wsh", [3, P, SH_F], F32, kind="ExternalInput")
    y = nc.dram_tensor("y", [TPC, H], F32, kind="ExternalOutput")

    with ExitStack() as ctx:
        tc = ctx.enter_context(tile.TileContext(nc))
        dram = ctx.enter_context(tc.tile_pool(name="dram", bufs=1, space="DRAM"))
        const = ctx.enter_context(tc.tile_pool(name="const", bufs=1))

        h_dram = dram.tile([I, TPC], F32)
        wdq_dram = dram.tile([I, H], BF16)
        swd_dram = dram.tile([1, TPC], F32)
        cc_in = dram.tile([1, 64], F32)
        cc_out = dram.tile([1, 64], F32)

        s_w = const.tile([P, 3], F32)       # weight scales (gate, up, down)
        neg_magic = const.tile([P, 1], F32)
        nc.vector.memset(neg_magic[:], -MAGIC)
        rs_w = const.tile([P, 3], F32)      # reciprocals
        swd_cols = const.tile([P, TPC // P], F32)  # s_h*ws_d, token-on-partition

        # ---------------- P0: weight scales ----------------
        def emit_p0(prep):
          with tc.tile_pool(name=f"p0{prep}", bufs=p0_bufs) as p0:
              partials = const.tile([P, 3, NCH, CHF // P], F32, tag="partials", name=f"partials{prep}")
              for w in range(3):
                  for k in range(NCH):
                      t = p0.tile([P, CHF], F32, tag="shard")
                      nc.sync.dma_start(t[:], wsh[w, :, k * CHF:(k + 1) * CHF])
                      nc.vector.tensor_reduce(
                          partials[:, w, k, :],
                          t[:].rearrange("p (a b) -> p a b", b=P),
                          axis=AX.X, op=ALU.add, apply_absolute_value=True,
                      )
              pw = const.tile([P, 3], F32, tag="pw", name=f"pw{prep}")
              for w in range(3):
                  nc.vector.tensor_reduce(
                      pw[:, w:w + 1], partials[:, w, :, :], axis=AX.XY, op=ALU.add)
              pr = const.tile([P, 3], F32, tag="pr", name=f"pr{prep}")
              nc.gpsimd.partition_all_reduce(pr[:], pw[:], P, bass_isa.ReduceOp.add)
              cc_row = const.tile([1, 64], F32, tag="cc_row", name=f"cc_row{prep}")
              nc.vector.memset(cc_row[:], 0.0)
              nc.vector.tensor_copy(cc_row[:, 0:3], pr[0:1, :])
              nc.gpsimd.dma_start(cc_in[:], cc_row[:])
              if single or "noCC" in phases:
                  nc.gpsimd.dma_start(cc_out[:], cc_in[:])
              else:
                  nc.gpsimd.collective_compute(
                      "AllReduce", ALU.add,
                      replica_groups=[list(range(NCORES))],
                      ins=[cc_in[:]], outs=[cc_out[:]],
                  )
              sumrow = const.tile([1, 64], F32, tag="sumrow", name=f"sumrow{prep}")
              nc.gpsimd.dma_start(sumrow[:], cc_out[:])
              sumb = const.tile([P, 64], F32, tag="sumb", name=f"sumb{prep}")
              nc.gpsimd.partition_broadcast(sumb[:], sumrow[:], P)
              # s_w = mean(|w|) + EPS ; rs_w = 1/s_w
              nc.vector.tensor_scalar(s_w[:], sumb[:, 0:3], INV_CNT, EPS, ALU.mult, ALU.add)
              nc.vector.reciprocal(rs_w[:], s_w[:])

        if not p0_in_body:
            emit_p0(0)
        for rep in range(reps):
            if p0_in_body:
                emit_p0(rep)
            # ---------------- pools that live through A+B ----------------
            with tc.tile_pool(name=f"ab{rep}", bufs=1) as ab:
                xq3 = ab.tile([P, HT, TPC], BF16)       # quantized x^T, resident
                xsws_g = ab.tile([P, TPC], F32)
                xsws_u = ab.tile([P, TPC], F32)
                m_h = ab.tile([P, TPC], F32)            # running absmax of h
                S_h = const.tile([P, TPC], F32, tag="S_h")

                # ---------------- A: quantize x (chunked so B starts early) ----------------
                with tc.tile_pool(name=f"pha{rep}", bufs=3) as pha:
                    m_x = pha.tile([P, TPC], F32, tag="m_x", bufs=1)
                    s_x = pha.tile([P, TPC], F32, tag="s_x", bufs=1)
                    r_x = pha.tile([P, TPC], F32, tag="r_x", bufs=1)
                    for ck in range(NB):
                        csl = bts(ck, 512)
                        xts = []
                        for ht in range(HT):
                            t = pha.tile([P, 512], F32, tag=f"xa{ht}", name=f"xa{ht}", bufs=2)
                            nc.sync.dma_start(t[:], xT[ht * P:(ht + 1) * P, csl])
                            xts.append(t)
                            if ht == 0:
                                nc.scalar.activation(m_x[:, csl], t[:], mybir.ActivationFunctionType.Abs)
                            else:
                                ta = pha.tile([P, 512], F32, tag="ta")
                                nc.scalar.activation(ta[:], t[:], mybir.ActivationFunctionType.Abs)
                                nc.vector.tensor_tensor(m_x[:, csl], m_x[:, csl], ta[:], ALU.max)
                        nc.gpsimd.partition_all_reduce(s_x[:, csl], m_x[:, csl], P, bass_isa.ReduceOp.absmax)
                        nc.vector.tensor_scalar(s_x[:, csl], s_x[:, csl], 1.0 / 127.0, EPS, ALU.mult, ALU.add)
                        nc.vector.reciprocal(r_x[:, csl], s_x[:, csl])
                        nc.vector.tensor_scalar(xsws_g[:, csl], s_x[:, csl], s_w[:, 0:1], None, ALU.mult)
                        nc.vector.tensor_scalar(xsws_u[:, csl], s_x[:, csl], s_w[:, 1:2], None, ALU.mult)
                        for ht in range(HT):
                            tmp = pha.tile([P, 512], F32, tag="xtmp")
                            nc.vector.tensor_tensor(tmp[:], xts[ht][:], r_x[:, csl], ALU.mult)
                            nc.vector.tensor_scalar(xq3[:, ht, csl], tmp[:], MAGIC, MAGIC, ALU.add, ALU.subtract)

                # ---------------- B: gate/up + h ----------------
                with (
                    tc.tile_pool(name=f"phb{rep}", bufs=2) as phb,
                    tc.tile_pool(name=f"phbq{rep}", bufs=3) as phbq,
                    tc.tile_pool(name=f"psumB{rep}", bufs=3, space="PSUM") as psumB,
                ):
                    for i0 in range(IT if "B" in phases else 0):
                        gb = phb.tile([P, H], F32, tag="gb")
                        nc.sync.dma_start(gb[:], wg[i0].rearrange("p a b -> p (a b)"))
                        ub = phb.tile([P, H], F32, tag="ub")
                        nc.sync.dma_start(ub[:], wu[i0].rearrange("p a b -> p (a b)"))
                        # ternary quantize: round(clip(w/s, -1, 1)) as bf16
                        gqb = phbq.tile([P, H], BF16, tag="gqb")
                        nc.vector.tensor_scalar(gb[:], gb[:], rs_w[:, 0:1], 1.0, ALU.mult, ALU.min)
                        nc.vector.tensor_scalar(gb[:], gb[:], -1.0, MAGIC, ALU.max, ALU.add)
                        uqb = phbq.tile([P, H], BF16, tag="uqb")
                        nc.vector.tensor_scalar(ub[:], ub[:], rs_w[:, 1:2], 1.0, ALU.mult, ALU.min)
                        nc.vector.tensor_scalar(ub[:], ub[:], -1.0, MAGIC, ALU.max, ALU.add)
                        if act_round:
                            nc.scalar.activation(gqb[:], gb[:], mybir.ActivationFunctionType.Identity, bias=neg_magic[:])
                            nc.scalar.activation(uqb[:], ub[:], mybir.ActivationFunctionType.Identity, bias=neg_magic[:])
                        else:
                            nc.vector.tensor_scalar(gqb[:], gb[:], MAGIC, None, ALU.subtract)
                            nc.vector.tensor_scalar(uqb[:], ub[:], MAGIC, None, ALU.subtract)
                        for tb in range(NB):
                            tsl = bts(tb, 512)
                            psg = psumB.tile([P, 512], F32, tag="psg")
                            for ht in range(HT):
                                nc.tensor.matmul(
                                    psg[:], gqb[:, bts(ht, P)], xq3[:, ht, tsl],
                                    start=(ht == 0), stop=(ht == HT - 1))
                            psu = psumB.tile([P, 512], F32, tag="psu")
                            for ht in range(HT):
                                nc.tensor.matmul(
                                    psu[:], uqb[:, bts(ht, P)], xq3[:, ht, tsl],
                                    start=(ht == 0), stop=(ht == HT - 1))
                            tg = phb.tile([P, 512], F32, tag="tg")
                            nc.vector.tensor_tensor(tg[:], psg[:], xsws_g[:, tsl], ALU.mult)
                            gs = phb.tile([P, 512], F32, tag="gs")
                            nc.scalar.activation(gs[:], tg[:], mybir.ActivationFunctionType.Silu)
                            tu = phb.tile([P, 512], F32, tag="tu")
                            nc.vector.tensor_tensor(tu[:], psu[:], xsws_u[:, tsl], ALU.mult)
                            hb = phb.tile([P, 512], F32, tag="hb")
                            nc.vector.tensor_tensor(hb[:], gs[:], tu[:], ALU.mult)
                            if i0 == 0:
                                nc.scalar.activation(m_h[:, tsl], hb[:], mybir.ActivationFunctionType.Abs)
                            else:
                                ha = phb.tile([P, 512], F32, tag="ha")
                                nc.scalar.activation(ha[:], hb[:], mybir.ActivationFunctionType.Abs)
                                nc.vector.tensor_tensor(m_h[:, tsl], m_h[:, tsl], ha[:], ALU.max)
                            nc.scalar.dma_start(h_dram[i0 * P:(i0 + 1) * P, tsl], hb[:])

                    # C0: quantize w_down -> bf16 scratch (fills engine gaps during B)
                    for i0 in range(IT if "C" in phases else 0):
                        db = phb.tile([P, H], F32, tag="db")
                        nc.sync.dma_start(db[:], wdT[i0 * P:(i0 + 1) * P, :])
                        dqb = phbq.tile([P, H], BF16, tag="dqb")
                        nc.vector.tensor_scalar(db[:], db[:], rs_w[:, 2:3], 1.0, ALU.mult, ALU.min)
                        nc.vector.tensor_scalar(db[:], db[:], -1.0, MAGIC, ALU.max, ALU.add)
                        nc.vector.tensor_scalar(dqb[:], db[:], MAGIC, None, ALU.subtract)
                        nc.scalar.dma_start(wdq_dram[i0 * P:(i0 + 1) * P, :], dqb[:])

                if "B" in phases:
                    nc.gpsimd.partition_all_reduce(S_h[:], m_h[:], P, bass_isa.ReduceOp.absmax)
                else:
                    nc.vector.memset(S_h[:], 1.0)

            # ---------------- C: down proj ----------------
            with (
                tc.tile_pool(name=f"phc{rep}", bufs=2) as phc,
                tc.tile_pool(name=f"hqp{rep}", bufs=2) as hqp,
                tc.tile_pool(name=f"psumC{rep}", bufs=2, space="PSUM") as psumC,
            ):
                s_h = phc.tile([P, TPC], F32, tag="s_h", bufs=1)
                nc.vector.tensor_scalar(s_h[:], S_h[:], 1.0 / 127.0, EPS, ALU.mult, ALU.add)
                r_h = phc.tile([P, TPC], F32, tag="r_h", bufs=1)
                nc.vector.reciprocal(r_h[:], s_h[:])
                swd = phc.tile([P, TPC], F32, tag="swd", bufs=1)
                nc.vector.tensor_scalar(swd[:], s_h[:], s_w[:, 2:3], None, ALU.mult)
                nc.scalar.dma_start(swd_dram[:], swd[0:1, :])
                nc.sync.dma_start(
                    swd_cols[:], swd_dram[:].rearrange("o (a p) -> p (o a)", p=P))

                for g in range(NB if "C" in phases else 0):
                    gsl = bts(g, 512)
                    hq = hqp.tile([P, IT, 512], BF16, tag="hq")
                    for i0 in range(IT):
                        hf = phc.tile([P, 512], F32, tag="hf")
                        nc.sync.dma_start(hf[:], h_dram[i0 * P:(i0 + 1) * P, gsl])
                        tmp = phc.tile([P, 512], F32, tag="hqt")
                        nc.vector.tensor_tensor(tmp[:], hf[:], r_h[:, gsl], ALU.mult)
                        nc.vector.tensor_scalar(hq[:, i0, :], tmp[:], MAGIC, MAGIC, ALU.add, ALU.subtract)
                    WD = 1024 if H % 1024 == 0 else 512
                    NJ2 = WD // 512
                    for hcp in range(H // WD):
                        pys = [psumC.tile([P, 512], F32, tag=f"py{j}", name=f"py{j}", bufs=1)
                               for j in range(4 * NJ2)]
                        for i0 in range(IT):
                            wdt = phc.tile([P, WD], BF16, tag="wdt", bufs=4)
                            nc.sync.dma_start(
                                wdt[:], wdq_dram[i0 * P:(i0 + 1) * P, hcp * WD:(hcp + 1) * WD])
                            for t4 in range(4):
                                for j2 in range(NJ2):
                                    nc.tensor.matmul(
                                        pys[j2 * 4 + t4][:], hq[:, i0, bts(t4, P)],
                                        wdt[:, bts(j2, 512)],
                                        start=(i0 == 0), stop=(i0 == IT - 1))
                        for j2 in range(NJ2):
                            for t4 in range(4):
                                tk = g * 4 + t4
                                ys = phc.tile([P, 512], F32, tag="ys", bufs=4)
                                nc.vector.tensor_scalar(
                                    ys[:], pys[j2 * 4 + t4][:], swd_cols[:, tk:tk + 1], None, ALU.mult)
                                nc.scalar.dma_start(
                                    y[tk * P:(tk + 1) * P, (hcp * NJ2 + j2) * 512:(hcp * NJ2 + j2 + 1) * 512], ys[:])


    nc.compile()
    return nc


def prep_inputs(x, w_gate, w_up, w_down):
    """Host-side shard/pack. Returns per-core input dicts."""
    BATCH = x.shape[0] * x.shape[1]
    H = x.shape[2]
    I = w_gate.shape[0]
    P = 128
    TPC = BATCH // NCORES
    x = np.ascontiguousarray(x, dtype=np.float32).reshape(BATCH, H)
    w_gate = np.ascontiguousarray(w_gate, dtype=np.float32)
    w_up = np.ascontiguousarray(w_up, dtype=np.float32)
    w_down = np.ascontiguousarray(w_down, dtype=np.float32)

    # [IT, P(h-in-tile), HT, P(i-in-tile)]: lhsT tiles, contiguous per i-tile
    def pack(w):
        return np.ascontiguousarray(
            w.reshape(I // P, P, H // P, P).transpose(0, 3, 2, 1))

    wg_p = pack(w_gate)
    wu_p = pack(w_up)
    wdT = np.ascontiguousarray(w_down.T)  # [I, H]

    ne = I * H // NCORES
    sh_f = ne // P
    in_maps = []
    for c in range(NCORES):
        xT_c = np.ascontiguousarray(x[c * TPC:(c + 1) * TPC].T)
        wsh_c = np.stack([
            w_gate.reshape(-1)[c * ne:(c + 1) * ne].reshape(P, sh_f),
            w_up.reshape(-1)[c * ne:(c + 1) * ne].reshape(P, sh_f),
            w_down.reshape(-1)[c * ne:(c + 1) * ne].reshape(P, sh_f),
        ])
        in_maps.append({
            "xT": xT_c, "wg": wg_p, "wu": wu_p, "wdT": wdT,
            "wsh": np.ascontiguousarray(wsh_c),
        })
    return in_maps


_CACHE = {}


def _get_nc(TPC, H, I):
    key = (TPC, H, I)
    if key not in _CACHE:
        _CACHE[key] = build_nc(TPC, H, I)
    return _CACHE[key]


def assemble_output(y_stack, out_shape):
    """y_stack: [NCORES, TPC, H] per-core outputs -> full [B, S, H]."""
    out = np.concatenate(list(y_stack), axis=0)
    return out.reshape(out_shape).astype(np.float32)


def kernel(x, w_gate, w_up, w_down):
    B, S, H = x.shape
    I = w_gate.shape[0]
    TPC = (B * S) // NCORES
    nc = _get_nc(TPC, H, I)
    in_maps = prep_inputs(x, w_gate, w_up, w_down)
    res = run_bass_kernel_spmd(nc, in_maps, core_ids=list(range(NCORES)))
    y = np.stack([res.results[c]["y"] for c in range(NCORES)])
    return assemble_output(y, (B, S, H))


if __name__ == "__main__":
    # tiny self-check against a numpy emulation of the reference
    rng = np.random.default_rng(0)
    B, S, H, I = 2, 2048, 512, 1024
    x = (rng.standard_normal((B, S, H)) * 1.0).astype(np.float32)
    wg = (rng.standard_normal((I, H)) / np.sqrt(H)).astype(np.float32)
    wu = (rng.standard_normal((I, H)) / np.sqrt(H)).astype(np.float32)
    wd = (rng.standard_normal((H, I)) / np.sqrt(I)).astype(np.float32)

    def qw(w):
        s = np.abs(w).mean(dtype=np.float64).astype(np.float32) + EPS
        return np.clip(np.round(w / s), -1, 1), s

    def qa(t):
        s = np.abs(t).max(-1, keepdims=True) / 127.0 + EPS
        return np.clip(np.round(t / s), -128, 127), s

    def ql(t, w):
        wq, ws = qw(w)
        tq, sx = qa(t)
        return (tq @ wq.T) * sx * ws

    def silu(v):
        return v / (1.0 + np.exp(-v))

    xx = x.reshape(-1, H).astype(np.float32)
    gate = silu(ql(xx, wg))
    up = ql(xx, wu)
    ref = ql((gate * up).astype(np.float32), wd).reshape(B, S, H)

    out = kernel(x, wg, wu, wd)
    num = np.abs(out - ref).max()
    den = np.abs(ref).max()
    l2 = np.linalg.norm(out - ref) / np.linalg.norm(ref)
    print(f"small test: absmax {num:.4g} (ref max {den:.4g}), rel-l2 {l2:.4g}")

